# revision 5
# baseline (speedup 1.0000x reference)
"""Trainium2 Bass kernel for nn_CircuitChannel (20-qubit statevector circuit).

Strategy: batch-parallel — BATCH=8 == n_cores, one full 2^20 statevector per
NeuronCore (8 MB fp32 re+im, resident in SBUF). Each of the 4 circuit layers
applies RX on all 20 qubits + a CZ-ring diagonal sign. Gates are applied as
7-qubit-group 128x128 complex matmuls on the TensorEngine:

 - TM stages ("transposing matmul"): stationary operand = a 128x128 state
   block, moving operand = the gate matrix; the result lands transposed in
   PSUM, which both applies the gate to the current partition-axis qubits
   AND swaps a 7-bit free-axis group onto the partition axis. Two window
   variants (TM0 / TM6) are block-swap involutions of the bit layout.
 - PM stages: stationary = gate matrix, moving = state columns; layout
   unchanged. The per-layer CZ diagonal (precomputed sign tensor in the
   current bit layout) is folded into the PM PSUM-evacuation as a
   tensor_tensor multiply (same 1x DVE cost as the copy it replaces).

Stage plan [TM0,TM6,PM]x2 then [TM6,TM0,PM]x2 covers all 20 qubits each
layer and returns the layout to canonical, so the terminal qubit-0
measurement is a partition-halved reduction and the output DMA is fully
contiguous. float32r (fp32-in-memory, full-rate PE mode, ~1.5e-4 L2 per
pass) is used for all matmuls.
"""
import sys
sys.path.insert(0, "/opt/trn_rl_repo")
import numpy as np

N = 20
DIM = 1 << N
BATCH = 8
NLAYERS = 4

STAGES = [
    ("TM6", 0), ("TM0", 0), ("PM", 0),
    ("TM6", 1), ("TM0", 1), ("PM", 1),
    ("TM0", 2), ("TM6", 2), ("PM", 2),
    ("TM0", 3), ("TM6", 3), ("PM", 3),
]


# ------------------------- host-side plan -------------------------

def _rx(theta):
    c, s = np.cos(theta / 2), np.sin(theta / 2)
    return np.array([[c, -1j * s], [-1j * s, c]], dtype=np.complex128)


def _cz_sign_canonical():
    idx = np.arange(DIM, dtype=np.int64)
    bits = (idx[None, :] >> (N - 1 - np.arange(N)[:, None])) & 1
    par = np.sum(bits[:-1] * bits[1:], axis=0) % 2
    return (1 - 2 * par).astype(np.float64)


def _apply_sigma(layout, t):
    l = list(layout)
    if t == 6:
        return l[13:20] + l[7:13] + l[0:7]
    return l[7:14] + l[0:7] + l[14:20]


def _sign_in_layout(s_canon, layout):
    pf = np.arange(DIM, dtype=np.int64)
    idx = np.zeros(DIM, dtype=np.int64)
    for j in range(N):
        bit = (pf >> (N - 1 - j)) & 1
        idx |= bit << (N - 1 - layout[j])
    return s_canon[idx].reshape(128, 8192).astype(np.float32)


def build_plan(thetas):
    s_canon = _cz_sign_canonical()
    layout = list(range(N))
    plan = []
    done = set()
    cur_layer = -1
    for stype, layer in STAGES:
        if layer != cur_layer:
            assert cur_layer == -1 or len(done) == N, (cur_layer, len(done))
            done = set()
            cur_layer = layer
        U = np.array([[1.0 + 0j]])
        for j in range(7):
            q = layout[j]
            g = np.eye(2, dtype=np.complex128) if q in done else _rx(thetas[layer, q])
            done.add(q)
            U = np.kron(U, g)
        st = dict(type=stype, U=U)
        if stype == "TM6":
            layout = _apply_sigma(layout, 6)
        elif stype == "TM0":
            layout = _apply_sigma(layout, 0)
        else:
            st["sign"] = _sign_in_layout(s_canon, layout)
        plan.append(st)
    assert len(done) == N
    assert layout == list(range(N))
    return plan


def stage_weights(plan):
    """Per-stage weight arrays. TM: [128,512] = [UrT|UiT|-UiT|UrT].
    PM: [128,384] = [UrT|-UiT|UiT]."""
    ws = []
    for st in plan:
        Ur = np.ascontiguousarray(st["U"].real.astype(np.float32))
        Ui = np.ascontiguousarray(st["U"].imag.astype(np.float32))
        if st["type"] == "PM":
            w = np.concatenate([Ur.T, -Ui.T, Ui.T], axis=1)
        else:
            w = np.concatenate([Ur.T, Ui.T, -Ui.T, Ur.T], axis=1)
        ws.append(np.ascontiguousarray(w.astype(np.float32)))
    return ws


# ------------------------- device program -------------------------

_NC_CACHE = {}


def _build_nc(reps=1):
    import concourse.bacc as bacc
    import concourse.mybir as mybir
    import concourse.tile as tile

    F32 = mybir.dt.float32
    F32R = mybir.dt.float32r
    AX = mybir.AluOpType
    ACTF = mybir.ActivationFunctionType

    nc = bacc.Bacc(None)
    pr = nc.declare_dram_parameter("pr", [128, 8192], F32R, isOutput=False)
    pi = nc.declare_dram_parameter("pi", [128, 8192], F32R, isOutput=False)
    wps = []
    for s, (stype, _) in enumerate(STAGES):
        shape = [128, 384] if stype == "PM" else [128, 512]
        wps.append(nc.declare_dram_parameter(f"w{s}", shape, F32R, isOutput=False))
    sgs = [nc.declare_dram_parameter(f"sg{l}", [128, 8192], F32, isOutput=False)
           for l in range(NLAYERS)]
    uvec = nc.declare_dram_parameter("uvec", [128, 1], F32, isOutput=False)
    maskA = nc.declare_dram_parameter("maskA", [128, 1], F32, isOutput=False)
    ones64 = nc.declare_dram_parameter("ones64", [64, 128], F32, isOutput=False)
    out = nc.declare_dram_parameter("out", [128, 16384], F32, isOutput=True)

    with tile.TileContext(nc) as tc:
        with (
            tc.tile_pool(name="st", bufs=1) as stp,
            tc.tile_pool(name="wp", bufs=2) as wp,
            tc.tile_pool(name="sgp", bufs=1) as sgp,
            tc.tile_pool(name="small", bufs=1) as smp,
            tc.tile_pool(name="pstm", bufs=4, space="PSUM") as pstm,
            tc.tile_pool(name="pspm", bufs=2, space="PSUM") as pspm,
        ):
            Af = stp.tile([128, 16384], F32R, tag="A")
            Bf = stp.tile([128, 16384], F32R, tag="B")
            A = Af[:].rearrange("p (c f) -> p c f", c=2)
            Bv = Bf[:].rearrange("p (c f) -> p c f", c=2)
            sgt = sgp.tile([128, 8192], F32, tag="sg")

            # load state (chunked so stage 0 can start early)
            for ch in range(8):
                sl = slice(ch * 1024, (ch + 1) * 1024)
                nc.sync.dma_start(A[:, 0, sl], pr[:, sl])
                nc.sync.dma_start(A[:, 1, sl], pi[:, sl])

            def tm_stage(src, dst, w, dve_mod=2):
                for pr_ in range(32):
                    p = pstm.tile([128, 512], F32, tag="tm")
                    for b in range(2):
                        blk = pr_ * 2 + b
                        xr = src[:, 0, blk * 128:(blk + 1) * 128]
                        xi = src[:, 1, blk * 128:(blk + 1) * 128]
                        hs = slice(b * 256, b * 256 + 256)
                        nc.tensor.matmul(p[:, hs], xr, w[:, 0:256],
                                         start=True, stop=False)
                        nc.tensor.matmul(p[:, hs], xi, w[:, 256:512],
                                         start=False, stop=True)
                    pv = p[:].rearrange("p (b c x) -> p b c x", b=2, c=2)
                    dv = dst[:, :, pr_ * 256:(pr_ + 1) * 256].rearrange(
                        "p c (b x) -> p b c x", b=2)
                    if pr_ % dve_mod == 0:
                        nc.vector.tensor_copy(dv, pv)
                    else:
                        nc.scalar.copy(dv, pv)

            def tm0_stage(src, dst, w, dve_mod=2):
                srcr = src[:, 0, :].rearrange("p (w l) -> p l w", l=64)
                srci = src[:, 1, :].rearrange("p (w l) -> p l w", l=64)
                dstv = dst.rearrange("p c (w l) -> p l c w", l=64)
                for pr_ in range(32):
                    p = pstm.tile([128, 512], F32, tag="tm")
                    for b in range(2):
                        blk = pr_ * 2 + b
                        hs = slice(b * 256, b * 256 + 256)
                        nc.tensor.matmul(p[:, hs], srcr[:, blk, :], w[:, 0:256],
                                         start=True, stop=False)
                        nc.tensor.matmul(p[:, hs], srci[:, blk, :], w[:, 256:512],
                                         start=False, stop=True)
                    pv = p[:].rearrange("p (b c x) -> p b c x", b=2, c=2)
                    dv = dstv[:, pr_ * 2:pr_ * 2 + 2, :, :]
                    if pr_ % dve_mod == 0:
                        nc.vector.tensor_copy(dv, pv)
                    else:
                        nc.scalar.copy(dv, pv)

            def pm_stage(src, dst, w, sg_ap, n_fused=10):
                deferred = []
                for ch in range(16):
                    sl = slice(ch * 512, (ch + 1) * 512)
                    pre = pspm.tile([128, 512], F32, tag="pmre")
                    pim = pspm.tile([128, 512], F32, tag="pmim")
                    xr = src[:, 0, sl]
                    xi = src[:, 1, sl]
                    nc.tensor.matmul(pre[:], w[:, 0:128], xr, start=True, stop=False)
                    nc.tensor.matmul(pre[:], w[:, 128:256], xi, start=False, stop=True)
                    nc.tensor.matmul(pim[:], w[:, 256:384], xr, start=True, stop=False)
                    nc.tensor.matmul(pim[:], w[:, 0:128], xi, start=False, stop=True)
                    if ch < n_fused:
                        nc.vector.tensor_tensor(dst[:, 0, sl], pre[:], sg_ap[:, sl],
                                                op=AX.mult)
                        nc.vector.tensor_tensor(dst[:, 1, sl], pim[:], sg_ap[:, sl],
                                                op=AX.mult)
                    else:
                        # ACT evacuates; DVE applies the sign afterwards,
                        # overlapping the next stage's early blocks.
                        nc.scalar.copy(dst[:, 0, sl], pre[:])
                        nc.scalar.copy(dst[:, 1, sl], pim[:])
                        deferred.append(sl)
                for sl in deferred:
                    nc.vector.tensor_tensor(dst[:, 0, sl], dst[:, 0, sl],
                                            sg_ap[:, sl], op=AX.mult)
                    nc.vector.tensor_tensor(dst[:, 1, sl], dst[:, 1, sl],
                                            sg_ap[:, sl], op=AX.mult)

            cur, nxt = A, Bv
            for _rep in range(reps):
                for s, (stype, layer) in enumerate(STAGES):
                    shape = [128, 384] if stype == "PM" else [128, 512]
                    wt = wp.tile(shape, F32R, tag="w")
                    nc.gpsimd.dma_start(wt[:], wps[s][:])
                    if stype == "PM":
                        nc.sync.dma_start(sgt[:], sgs[layer][:])
                        pm_stage(cur, nxt, wt, sgt)
                    elif stype == "TM6":
                        tm_stage(cur, nxt, wt)
                    else:
                        tm0_stage(cur, nxt, wt)
                    cur, nxt = nxt, cur
            assert cur is A  # final state in A; B free for output staging

            # ---- measurement on qubit 0 (= partition MSB; partitions 0..63)
            acc = smp.tile([64, 4], F32, tag="acc")
            scr_r = Bf[0:64, 0:8192]
            scr_i = Bf[0:64, 8192:16384]
            nc.scalar.activation(scr_r, A[0:64, 0, :], ACTF.Square,
                                 accum_out=acc[:, 0:1])
            nc.vector.scalar_tensor_tensor(scr_i, A[0:64, 1, :], 1.0,
                                           A[0:64, 1, :], op0=AX.bypass,
                                           op1=AX.mult, accum_out=acc[:, 1:2])
            nc.vector.tensor_add(acc[:, 2:3], acc[:, 0:1], acc[:, 1:2])
            o64 = smp.tile([64, 128], F32, tag="ones")
            nc.gpsimd.dma_start(o64[:], ones64[:])
            pp0 = pstm.tile([128, 1], F32, tag="tm")
            nc.tensor.matmul(pp0[:], o64[:], acc[:, 2:3], start=True, stop=True)

            sm = smp.tile([128, 12], F32, tag="sm")
            p0v, tv, a1, a2, pv_, rv, invv, omt, s0, s1, diff, S = (
                sm[:, k:k + 1] for k in range(12))
            uvt = smp.tile([128, 1], F32, tag="uv")
            mAt = smp.tile([128, 1], F32, tag="mA")
            nc.gpsimd.dma_start(uvt[:], uvec[:])
            nc.gpsimd.dma_start(mAt[:], maskA[:])
            nc.vector.tensor_copy(p0v, pp0[:])
            nc.vector.tensor_tensor(tv, uvt[:], p0v, op=AX.is_ge)
            nc.vector.tensor_scalar(a1, p0v, -2.0, 1.0, op0=AX.mult, op1=AX.add)
            nc.vector.tensor_tensor(a2, tv, a1, op=AX.mult)
            nc.vector.tensor_tensor(pv_, p0v, a2, op=AX.add)
            nc.vector.reciprocal(rv, pv_)
            nc.scalar.sqrt(invv, rv)
            nc.vector.tensor_scalar(omt, tv, -1.0, 1.0, op0=AX.mult, op1=AX.add)
            nc.vector.tensor_tensor(s0, invv, omt, op=AX.mult)
            nc.vector.tensor_tensor(s1, invv, tv, op=AX.mult)
            nc.vector.tensor_tensor(diff, s0, s1, op=AX.subtract)
            nc.vector.tensor_tensor(a2, mAt[:], diff, op=AX.mult)
            nc.vector.tensor_tensor(S, s1, a2, op=AX.add)

            # ---- interleave re/im with scale, then DMA out
            Bpair = Bf[:].rearrange("p (f c) -> p f c", c=2)
            for ch in range(8):
                fsl = slice(ch * 1024, (ch + 1) * 1024)
                nc.vector.tensor_scalar(Bpair[:, fsl, 0], A[:, 0, fsl], S, None,
                                        op0=AX.mult)
                nc.scalar.mul(Bpair[:, fsl, 1], A[:, 1, fsl], S)
                osl = slice(ch * 2048, (ch + 1) * 2048)
                nc.sync.dma_start(out[:, osl], Bf[:, osl].bitcast(F32))
    nc.compile()
    return nc


def _get_nc(reps=1):
    if reps not in _NC_CACHE:
        _NC_CACHE[reps] = _build_nc(reps)
    return _NC_CACHE[reps]


# ------------------------- entry point -------------------------

def kernel(psi_re, psi_im, thetas, u, _trace=False):
    from concourse.bass_utils import run_bass_kernel_spmd

    psi_re = np.ascontiguousarray(np.asarray(psi_re, dtype=np.float32))
    psi_im = np.ascontiguousarray(np.asarray(psi_im, dtype=np.float32))
    thetas = np.asarray(thetas, dtype=np.float32)
    u = np.asarray(u, dtype=np.float32)

    plan = build_plan(thetas.astype(np.float64))
    ws = stage_weights(plan)
    signs = [st["sign"] for st in plan if st["type"] == "PM"]
    maskA = (np.arange(128) < 64).astype(np.float32).reshape(128, 1)
    ones64 = np.ones((64, 128), dtype=np.float32)

    nc = _get_nc()
    in_maps = []
    for b in range(BATCH):
        m = {
            "pr": psi_re[b].reshape(128, 8192),
            "pi": psi_im[b].reshape(128, 8192),
            "uvec": np.full((128, 1), u[b], dtype=np.float32),
            "maskA": maskA,
            "ones64": ones64,
        }
        for s in range(len(STAGES)):
            m[f"w{s}"] = ws[s]
        for l in range(NLAYERS):
            m[f"sg{l}"] = signs[l]
        in_maps.append(m)

    res = run_bass_kernel_spmd(nc, in_maps, list(range(BATCH)), trace=_trace)
    outs = np.stack([res.results[b]["out"].reshape(DIM, 2) for b in range(BATCH)])
    if _trace:
        return outs, res
    return outs
